# revision 1
# baseline (speedup 1.0000x reference)
"""Trainium2 Bass kernel for nn_DriftRectifier (2-block Mamba over 64x64 images).

Sharding: data-parallel over batch B=16 -> 2 samples per core x 8 cores.
v2 architecture (engine-balanced around the DVE scan floor):
  - Vector: the 16-n selective scans (tensor_tensor_scan, DVE-only op),
    most hc multiplies, small [<=64,512] ops.
  - GpSimd: dbu multiplies (dtu*B), dtu, yo gate, some hc.
  - Scalar (ACT): exp/silu/softplus, PSUM->SBUF copies; exp/ln batched and
    the whole post phase kept inside the sqrt table set (sqrt/square/copy/
    identity coexist) to avoid ACT table thrash.
  - PE: projections, per-n y accumulation via identity matmuls, LN
    mean row-sums and [1->64] broadcasts (replaces DRAM round-trips).
  - DMA: B/C rows partition-broadcast from a DRAM staging tile; B and C
    fetched in ONE descriptor per (n, half) via a 3-dim access pattern.
Units (sample, block) are pipelined: u/zs/dtu/dt tiles are parity-
duplicated so unit k+1's projections overlap unit k's scan.
"""
import contextlib

import numpy as np

B, C, H, W = 16, 4, 64, 64
L = H * W  # 4096
DM, DI, DS, DK, DR = 64, 128, 16, 4, 4
NCORES = 8
BPC = B // NCORES  # samples per core
TC = 512           # psum / matmul chunk
NCH = L // TC      # 8 chunks
HALF = L // 2      # 2048, scan half-sequence
EPS = 1e-5

# hc (h * C) engine split: n-values in VHC run on vector, rest on gpsimd
VHC = frozenset(range(DS))  # all on vector: gpsimd blocks DVE via the shared SBUF port pair

_CACHE = {}


def _build_program():
    import concourse.bacc as bacc
    import concourse.bass as bass
    from concourse import mybir
    from concourse.tile import TileContext

    F32 = mybir.dt.float32
    BF16 = mybir.dt.bfloat16
    AF = mybir.ActivationFunctionType
    OP = mybir.AluOpType

    nc = bacc.Bacc("TRN2")

    # ---- dram I/O ----
    zc = nc.dram_tensor("zc", [BPC, C, L], F32, kind="ExternalInput")
    out = nc.dram_tensor("out", [BPC, C, L], F32, kind="ExternalOutput")
    ident_in = nc.dram_tensor("ident", [128, 128], BF16, kind="ExternalInput")
    emb_wT = nc.dram_tensor("emb_wT", [C, DM], F32, kind="ExternalInput")
    emb_b = nc.dram_tensor("emb_b", [DM, 1], F32, kind="ExternalInput")
    head_wT = nc.dram_tensor("head_wT", [DM, C], BF16, kind="ExternalInput")
    neg_head_b = nc.dram_tensor("neg_head_b", [C, 1], F32, kind="ExternalInput")
    onehot4_in = nc.dram_tensor("onehot4", [4, 4 * DM], F32, kind="ExternalInput")
    wsel4_in = nc.dram_tensor("wsel4", [DM, 16], BF16, kind="ExternalInput")
    blk_t = []
    for m in (1, 2):
        p = f"m{m}_"
        blk_t.append({
            "cwu0": nc.dram_tensor(p + "cwu0", [2 * DM, DI], BF16, kind="ExternalInput"),
            "cwu1": nc.dram_tensor(p + "cwu1", [2 * DM, DI], BF16, kind="ExternalInput"),
            "inw_zT": nc.dram_tensor(p + "inw_zT", [DM, DI], BF16, kind="ExternalInput"),
            "conv_b": nc.dram_tensor(p + "conv_b", [DI, 1], F32, kind="ExternalInput"),
            "xpwT": nc.dram_tensor(p + "xpwT", [DI, DR + 2 * DS], BF16, kind="ExternalInput"),
            "dtpwT": nc.dram_tensor(p + "dtpwT", [DR, DI], BF16, kind="ExternalInput"),
            "dtp_b": nc.dram_tensor(p + "dtp_b", [DI, 1], F32, kind="ExternalInput"),
            "A": nc.dram_tensor(p + "A", [DI, DS], F32, kind="ExternalInput"),
            "D": nc.dram_tensor(p + "D", [DI, 1], F32, kind="ExternalInput"),
            "opwT": nc.dram_tensor(p + "opwT", [DI, DM], BF16, kind="ExternalInput"),
            "ln_g": nc.dram_tensor(p + "ln_g", [DM, 1], F32, kind="ExternalInput"),
            "ln_b": nc.dram_tensor(p + "ln_b", [DM, 1], F32, kind="ExternalInput"),
        })

    with TileContext(nc) as tc, contextlib.ExitStack() as ctx:
        consts = ctx.enter_context(tc.tile_pool(name="consts", bufs=1))
        persist = ctx.enter_context(tc.tile_pool(name="persist", bufs=1))
        bcw = ctx.enter_context(tc.tile_pool(name="bcw", bufs=3))
        enw = ctx.enter_context(tc.tile_pool(name="enw", bufs=3))
        nwork = ctx.enter_context(tc.tile_pool(name="nwork", bufs=3))
        small = ctx.enter_context(tc.tile_pool(name="small", bufs=2))
        stp = ctx.enter_context(tc.tile_pool(name="stp", bufs=1))
        psA = ctx.enter_context(tc.tile_pool(name="psA", bufs=2, space="PSUM"))
        psB = ctx.enter_context(tc.tile_pool(name="psB", bufs=1, space="PSUM"))
        psY = ctx.enter_context(tc.tile_pool(name="psY", bufs=1, space="PSUM"))
        dstage = ctx.enter_context(tc.tile_pool(name="dstage", bufs=4, space="DRAM"))

        # ---- constants to SBUF ----
        ident = consts.tile([128, 128], BF16)
        nc.sync.dma_start(out=ident, in_=ident_in[:])
        sb_embT = consts.tile([C, DM], F32)
        nc.sync.dma_start(out=sb_embT, in_=emb_wT[:])
        sb_embb = consts.tile([DM, 1], F32)
        nc.sync.dma_start(out=sb_embb, in_=emb_b[:])
        sb_headT = consts.tile([DM, C], BF16)
        nc.sync.dma_start(out=sb_headT, in_=head_wT[:])
        sb_nhb = consts.tile([C, 1], F32)
        nc.sync.dma_start(out=sb_nhb, in_=neg_head_b[:])
        # LN helpers: column of 1/64 (mean weights), row of ones (broadcast)
        eps8 = consts.tile([NCH, 1], F32)
        nc.vector.memset(eps8, EPS)
        oh4 = consts.tile([4, 4 * DM], F32)
        nc.sync.dma_start(out=oh4, in_=onehot4_in[:])
        wsel4 = consts.tile([DM, 16], BF16)
        nc.sync.dma_start(out=wsel4, in_=wsel4_in[:])
        one128 = consts.tile([DI, 1], F32)
        nc.vector.memset(one128, 1.0)
        blk = []
        for m in range(2):
            d = {}
            for k, t in blk_t[m].items():
                d[k] = consts.tile(list(t.shape), t.dtype, name=f"c_m{m}_{k}")
                nc.sync.dma_start(out=d[k], in_=t[:])
            blk.append(d)

        # ---- persistent tiles ----
        # feat2x: shared across units (k-post writes unit-(k+1) input after
        # k-proj has consumed it; WAR tracked by the tile framework)
        feat2x = persist.tile([2 * DM, L + 3], BF16)
        # parity-duplicated so unit k+1's proj can overlap unit k's scan/post
        u_bf = [persist.tile([DI, L], BF16, name=f"u{i}") for i in range(2)]
        zs_bf = [persist.tile([DI, L], BF16, name=f"zs{i}") for i in range(2)]
        dtu_bf = [persist.tile([DI, L], BF16, name=f"dtu{i}") for i in range(2)]
        dt_f32 = [persist.tile([DI, L], BF16, name=f"dt{i}") for i in range(2)]
        yo_bf = persist.tile([DI, L], BF16)
        fch_bf = persist.tile([DM, L], BF16)
        carry = persist.tile([DI, DS], F32)

        def emit_embed(si):
            with nc.named_scope(f"s{si}_embed"):
                for c in range(NCH):
                    cs = slice(c * TC, (c + 1) * TC)
                    zch = small.tile([C, TC], F32, name="zch", tag="zch")
                    nc.scalar.dma_start(out=zch, in_=zc[si][:, cs])
                    ps = psA.tile([DM, TC], F32, name="emb_ps", tag="mm")
                    nc.tensor.matmul(ps, lhsT=sb_embT, rhs=zch,
                                     start=True, stop=True)
                    nc.scalar.activation(
                        out=feat2x[0:DM, 3 + c * TC:3 + (c + 1) * TC],
                        in_=ps, func=AF.Identity, bias=sb_embb[:, :])
                    nc.scalar.activation(
                        out=feat2x[DM:2 * DM, 2 + c * TC:2 + (c + 1) * TC],
                        in_=ps, func=AF.Identity, bias=sb_embb[:, :])
                nc.vector.memset(feat2x[0:DM, 0:3], 0.0)
                nc.vector.memset(feat2x[DM:2 * DM, 0:2], 0.0)

        emit_embed(0)
        for s in range(BPC):
            for m in range(2):
                w = blk[m]
                par = (2 * s + m) % 2
                u_t, zs_t, dtu_t, dt_t = u_bf[par], zs_bf[par], dtu_bf[par], dt_f32[par]
                bc_dram = dstage.tile([2 * DS, L], BF16, name="bc_dram")

                with nc.named_scope(f"s{s}m{m}_proj"):
                    # silu pass (conv fused into in_proj via shifted feat2x)
                    for c in range(NCH):
                        cs = slice(c * TC, (c + 1) * TC)
                        ups = psA.tile([DI, TC], F32, name="ups", tag="mm")
                        nc.tensor.matmul(ups, lhsT=w["cwu0"],
                                         rhs=feat2x[:, c * TC:c * TC + TC],
                                         start=True, stop=False)
                        nc.tensor.matmul(ups, lhsT=w["cwu1"],
                                         rhs=feat2x[:, c * TC + 2:c * TC + 2 + TC],
                                         start=False, stop=True)
                        nc.scalar.activation(out=u_t[:, cs], in_=ups, func=AF.Silu,
                                             bias=w["conv_b"][:, :])
                        zps = psA.tile([DI, TC], F32, name="zps", tag="mm")
                        nc.tensor.matmul(zps, lhsT=w["inw_zT"],
                                         rhs=feat2x[0:DM, 3 + c * TC:3 + (c + 1) * TC],
                                         start=True, stop=True)
                        nc.scalar.activation(out=zs_t[:, cs], in_=zps, func=AF.Silu)
                    # x_proj / dt pass: exps batched per half, then one ln
                    for half in range(2):
                        spe = enw.tile([DI, HALF], F32, name="spe", tag="en")
                        for cc in range(NCH // 2):
                            c = half * (NCH // 2) + cc
                            cs = slice(c * TC, (c + 1) * TC)
                            xps = psA.tile([DR + 2 * DS, TC], F32, name="xps", tag="mm")
                            nc.tensor.matmul(xps, lhsT=w["xpwT"], rhs=u_t[:, cs],
                                             start=True, stop=True)
                            # x_proj rows host-permuted to [B(16), C(16), dt(4)]
                            bcc = small.tile([2 * DS, TC], BF16, name="bcc", tag="bcc")
                            nc.scalar.activation(out=bcc, in_=xps[0:2 * DS, :],
                                                 func=AF.Copy)
                            nc.sync.dma_start(out=bc_dram[:, cs], in_=bcc)
                            dtr = small.tile([DR, TC], BF16, name="dtr", tag="dtr")
                            nc.scalar.activation(out=dtr,
                                                 in_=xps[2 * DS:2 * DS + DR, :],
                                                 func=AF.Copy)
                            dtps = psA.tile([DI, TC], F32, name="dtps", tag="mm")
                            nc.tensor.matmul(dtps, lhsT=w["dtpwT"], rhs=dtr,
                                             start=True, stop=True)
                            # softplus(x) = ln(1 + exp(x))
                            nc.scalar.activation(out=spe[:, cc * TC:(cc + 1) * TC],
                                                 in_=dtps, func=AF.Exp,
                                                 bias=w["dtp_b"][:, :])
                        hsl = slice(half * HALF, (half + 1) * HALF)
                        nc.scalar.activation(out=dt_t[:, hsl],
                                             in_=spe, func=AF.Ln, bias=one128[:, :])
                        for j in range(2):
                            qj = slice(half * HALF + j * (HALF // 2),
                                       half * HALF + (j + 1) * (HALF // 2))
                            nc.vector.tensor_tensor(out=dtu_t[:, qj], in0=dt_t[:, qj],
                                                    in1=u_t[:, qj], op=OP.mult)

                if m == 1 and s + 1 < BPC:
                    emit_embed(s + 1)

                def emit_post_half(h2):
                    with nc.named_scope(f"s{s}m{m}_post{h2}"):
                        mps4 = psB.tile([4, TC], F32, name="mps4", tag="mps4")
                        msp4 = psB.tile([4, TC], F32, name="msp4", tag="msp4")
                        for cc in range(4):
                            c = h2 * 4 + cc
                            cs = slice(c * TC, (c + 1) * TC)
                            fps = psA.tile([DM, TC], F32, name="fps", tag="mm")
                            nc.tensor.matmul(fps, lhsT=w["opwT"], rhs=yo_bf[:, cs],
                                             start=True, stop=True)
                            nc.scalar.activation(out=fch_bf[:, cs], in_=fps,
                                                 func=AF.Copy)
                            sq = small.tile([DM, TC], BF16, name="sq", tag="sq")
                            nc.scalar.activation(out=sq, in_=fch_bf[:, cs],
                                                 func=AF.Square)
                            wsl = wsel4[:, cc * 4:(cc + 1) * 4]
                            nc.tensor.matmul(mps4, lhsT=wsl, rhs=fch_bf[:, cs],
                                             start=(cc == 0), stop=(cc == 3))
                            nc.tensor.matmul(msp4, lhsT=wsl, rhs=sq,
                                             start=(cc == 0), stop=(cc == 3))
                        mu4 = stp.tile([4, TC], F32, name="mu4", tag="mu8")
                        nc.scalar.activation(out=mu4, in_=mps4, func=AF.Copy)
                        mu2_4 = stp.tile([4, TC], F32, name="mu2_4", tag="mu2_8")
                        nc.vector.tensor_tensor(out=mu2_4, in0=mu4, in1=mu4,
                                                op=OP.mult)
                        var4 = stp.tile([4, TC], F32, name="var4", tag="var8")
                        nc.vector.tensor_tensor(out=var4, in0=msp4, in1=mu2_4,
                                                op=OP.subtract)
                        sd4 = stp.tile([4, TC], F32, name="sd4", tag="mu2_8")
                        nc.scalar.activation(out=sd4, in_=var4, func=AF.Sqrt,
                                             bias=eps8[0:4, :])
                        rstd4 = stp.tile([4, TC], F32, name="rstd4", tag="var8")
                        nc.vector.reciprocal_approx_fast(out=rstd4, in_=sd4)
                        for cc in range(4):
                            c = h2 * 4 + cc
                            cs = slice(c * TC, (c + 1) * TC)
                            mbc = psA.tile([DM, TC], F32, name="mbc", tag="mm")
                            nc.tensor.matmul(mbc, lhsT=oh4[:, cc * DM:(cc + 1) * DM],
                                             rhs=mu4, start=True, stop=True)
                            rbc = psA.tile([DM, TC], F32, name="rbc", tag="mm")
                            nc.tensor.matmul(rbc, lhsT=oh4[:, cc * DM:(cc + 1) * DM],
                                             rhs=rstd4, start=True, stop=True)
                            t1 = small.tile([DM, TC], BF16, name="t1", tag="t1")
                            nc.vector.tensor_tensor(out=t1, in0=fch_bf[:, cs],
                                                    in1=mbc, op=OP.subtract)
                            t2 = small.tile([DM, TC], BF16, name="t2", tag="t2")
                            nc.vector.tensor_tensor(out=t2, in0=t1, in1=rbc,
                                                    op=OP.mult)
                            if m == 0:
                                nc.scalar.activation(
                                    out=feat2x[0:DM, 3 + c * TC:3 + (c + 1) * TC],
                                    in_=t2, func=AF.Identity,
                                    scale=w["ln_g"][:, :], bias=w["ln_b"][:, :])
                                nc.scalar.activation(
                                    out=feat2x[DM:2 * DM, 2 + c * TC:2 + (c + 1) * TC],
                                    in_=t2, func=AF.Identity,
                                    scale=w["ln_g"][:, :], bias=w["ln_b"][:, :])
                            else:
                                # head input staged in a temp tile so the next
                                # sample's embed never waits on feat2x
                                hd = small.tile([DM, TC], BF16, name="hd", tag="hd")
                                nc.scalar.activation(
                                    out=hd, in_=t2, func=AF.Identity,
                                    scale=w["ln_g"][:, :], bias=w["ln_b"][:, :])
                                dps = psA.tile([C, TC], F32, name="dps", tag="mm")
                                nc.tensor.matmul(dps, lhsT=sb_headT, rhs=hd,
                                                 start=True, stop=True)
                                nd = small.tile([C, TC], F32, name="nd", tag="nd")
                                nc.scalar.activation(out=nd, in_=dps,
                                                     func=AF.Identity,
                                                     scale=-1.0, bias=sb_nhb[:, :])
                                zch2 = small.tile([C, TC], F32, name="zch2",
                                                  tag="zch")
                                nc.scalar.dma_start(out=zch2, in_=zc[s][:, cs])
                                oc = small.tile([C, TC], F32, name="oc", tag="ytmp")
                                nc.vector.tensor_tensor(out=oc, in0=zch2, in1=nd,
                                                        op=OP.add)
                                nc.gpsimd.dma_start(out=out[s][:, cs], in_=oc)



                with nc.named_scope(f"s{s}m{m}_scan"):
                    for q in range(2):
                        hs = q * HALF
                        qsl = slice(hs, hs + HALF)
                        yps = [psY.tile([DI, TC], F32, name=f"yps{k}", tag=f"yps{k}")
                               for k in range(HALF // TC)]
                        for n in range(DS):
                            en = enw.tile([DI, HALF], F32, name="en", tag="en")
                            nc.scalar.activation(out=en, in_=dt_t[:, qsl],
                                                 func=AF.Exp,
                                                 scale=w["A"][:, n:n + 1])
                            bc_t = bcw.tile([DI, 2 * HALF], BF16, name="bc_t",
                                            tag="bc_t")
                            nc.sync.dma_start(out=bc_t, in_=bass.AP(
                                tensor=bc_dram.tensor,
                                offset=bc_dram.offset + n * L + hs,
                                ap=[[0, DI], [DS * L, 2], [1, HALF]]))
                            dbu = nwork.tile([DI, HALF], BF16, name="dbu", tag="dbu")
                            nc.vector.tensor_tensor(out=dbu, in0=dtu_t[:, qsl],
                                                    in1=bc_t[:, 0:HALF], op=OP.mult)
                            h_t = nwork.tile([DI, HALF], BF16, name="h_t", tag="h_t")
                            init = 0.0 if q == 0 else carry[:, n:n + 1]
                            nc.vector.tensor_tensor_scan(
                                out=h_t, data0=en, data1=dbu,
                                initial=init, op0=OP.mult, op1=OP.add)
                            if q == 0:
                                nc.vector.tensor_copy(out=carry[:, n:n + 1],
                                                      in_=h_t[:, HALF - 1:HALF])
                            hc = nwork.tile([DI, HALF], BF16, name="hc", tag="hc")
                            heng = nc.vector if n in VHC else nc.gpsimd
                            heng.tensor_tensor(out=hc, in0=h_t,
                                               in1=bc_t[:, HALF:2 * HALF], op=OP.mult)
                            for k in range(HALF // TC):
                                nc.tensor.matmul(yps[k], lhsT=ident,
                                                 rhs=hc[:, k * TC:(k + 1) * TC],
                                                 start=(n == 0), stop=(n == DS - 1))
                        for k in range(HALF // TC):
                            cs = slice(hs + k * TC, hs + (k + 1) * TC)
                            tmp = small.tile([DI, TC], F32, name="ytmp", tag="ytmp")
                            nc.vector.scalar_tensor_tensor(
                                out=tmp, in0=u_t[:, cs], scalar=w["D"][:, :],
                                in1=yps[k], op0=OP.mult, op1=OP.add)
                            nc.vector.tensor_tensor(out=yo_bf[:, cs], in0=tmp,
                                                    in1=zs_t[:, cs], op=OP.mult)
                        emit_post_half(q)

    nc.finalize()
    return nc


def _prep_maps(inputs):
    import ml_dtypes
    bf = ml_dtypes.bfloat16
    f = np.float32
    z = np.asarray(inputs["z_damaged"], dtype=f).reshape(B, C, L)

    base = {
        "ident": np.eye(128, dtype=bf),
        "emb_wT": np.ascontiguousarray(np.asarray(inputs["emb_w"], f).T),
        "emb_b": np.asarray(inputs["emb_b"], f).reshape(DM, 1),
        "head_wT": np.ascontiguousarray(np.asarray(inputs["head_w"], f).T).astype(bf),
        "neg_head_b": (-np.asarray(inputs["head_b"], f)).reshape(C, 1),
        "onehot4": np.concatenate(
            [np.eye(4, dtype=f)[:, c:c + 1] * np.ones((1, DM), f)
             for c in range(4)], axis=1),
        "wsel4": np.concatenate(
            [np.eye(4, dtype=f)[c:c + 1, :] * np.full((DM, 1), 1.0 / DM, f)
             for c in range(4)], axis=1).astype(bf),
    }
    for m in (1, 2):
        p = f"m{m}_"
        inw = np.asarray(inputs[p + "in_proj_w"], f)  # [2DI, DM]
        w_u = inw[:DI]  # [DI, DM]
        cw = np.asarray(inputs[p + "conv_w"], f).reshape(DI, DK)
        # lhsT rows (k,m) -> cols d: w[d,k]*W_u[d,m]
        base[p + "cwu0"] = np.ascontiguousarray(np.concatenate(
            [cw[:, 0][None, :] * w_u.T, cw[:, 1][None, :] * w_u.T], axis=0)).astype(bf)
        base[p + "cwu1"] = np.ascontiguousarray(np.concatenate(
            [cw[:, 2][None, :] * w_u.T, cw[:, 3][None, :] * w_u.T], axis=0)).astype(bf)
        base[p + "inw_zT"] = np.ascontiguousarray(inw[DI:].T).astype(bf)
        base[p + "conv_b"] = np.asarray(inputs[p + "conv_b"], f).reshape(DI, 1)
        xpw = np.asarray(inputs[p + "x_proj_w"], f)  # rows: dt(4), B(16), C(16)
        xpw = np.concatenate([xpw[DR:], xpw[:DR]], axis=0)  # -> B, C, dt
        base[p + "xpwT"] = np.ascontiguousarray(xpw.T).astype(bf)
        base[p + "dtpwT"] = np.ascontiguousarray(
            np.asarray(inputs[p + "dt_proj_w"], f).T).astype(bf)
        base[p + "dtp_b"] = np.asarray(inputs[p + "dt_proj_b"], f).reshape(DI, 1)
        base[p + "A"] = -np.exp(np.asarray(inputs[p + "A_log"], f))
        base[p + "D"] = np.asarray(inputs[p + "D"], f).reshape(DI, 1)
        base[p + "opwT"] = np.ascontiguousarray(
            np.asarray(inputs[p + "out_proj_w"], f).T).astype(bf)
        base[p + "ln_g"] = np.asarray(inputs[f"ln{m}_g"], f).reshape(DM, 1)
        base[p + "ln_b"] = np.asarray(inputs[f"ln{m}_b"], f).reshape(DM, 1)

    maps = []
    for k in range(NCORES):
        mkp = dict(base)
        mkp["zc"] = np.ascontiguousarray(z[k * BPC:(k + 1) * BPC])
        maps.append(mkp)
    return maps


def _run(inputs, trace=False):
    from concourse.bass_utils import run_bass_kernel_spmd
    if "nc" not in _CACHE:
        _CACHE["nc"] = _build_program()
    nc = _CACHE["nc"]
    maps = _prep_maps(inputs)
    res = run_bass_kernel_spmd(nc, maps, core_ids=list(range(NCORES)), trace=trace)
    outs = [r["out"] for r in res.results]
    full = np.concatenate(outs, axis=0).reshape(B, C, H, W)
    return full, res


def kernel(**inputs):
    full, _ = _run(inputs, trace=False)
    return full

